# revision 14
# baseline (speedup 1.0000x reference)
"""Trainium2 Bass kernel for DifferentialEntropyRegularization (kNN loss).

reference math:
    dots = x @ x.T ; dots[i,i] = -1
    I = argmax(dots, axis=1)
    rho = ||x - x[I] + 1e-6||_2
    loss = -mean(log(rho + 1e-8))

Strategy (8 NeuronCores, data-parallel over rows of x, fp8 AllGather):
  - each core owns a 1024-row slab of queries. It PE-transposes ONLY its
    own slab to fp8 and AllGathers the transposed slabs (+2 fp8 rows
    packing -(||x_k||^2-512)/2 per key) across the 8 cores — no core
    transposes the full x, and x itself is never broadcast.
  - scores s[k,q] = <x_k,x_q>_fp8 - (n_k-512)/2 are computed KEYS-ON-
    PARTITIONS via fp8 DoubleRow matmuls; the per-key norm correction is
    a per-partition bias applied for free on the PSUM->SBUF copy.
  - nearest neighbor by squared distance: since s_self - s_cross =
    ||x_q - x_k||^2 / 2, we only need the top-2 of s per query:
    rho^2 = 2*(m1 - m2) with m1 = n_q/2 + 256 exact in fp32.
  - selection runs as an elementwise f16 running max over key-tiles
    (copies split ACT/DVE, max-accumulate split DVE/gpsimd in two
    independent chains), then a tiny PE transpose + MAX8 recovers the
    per-query top-2 across the 128 key lanes. The self-score dominates
    its lane, so the duplicated own-block (slot 0 = local copy, one AG
    chunk = same bytes) merges into one lane and rank-2 is the NN.
  - per-core partial sums of log(rho^2) reduced on host.
"""

import sys

sys.path.insert(0, "/opt/trn_rl_repo")

import numpy as np

import concourse.bass as bass
import concourse.mybir as mybir
import concourse.tile as tile
from concourse import bacc
from concourse.bass_utils import run_bass_kernel_spmd
from concourse.masks import make_identity

N = 8192
D = 512
NC = 8
SLAB = N // NC          # 1024 query rows per core
P = 128                 # partitions
QT = SLAB // P          # 8 row tiles per slab
KC = D // P             # 4 contraction chunks of 128
NSLOT = NC + 1          # 8 AG chunks + 1 local duplicate
PROWS = D + 2           # payload rows: 512 xT + 2 norm-correction rows

F32 = mybir.dt.float32
F16 = mybir.dt.float16
F8 = mybir.dt.float8e4
AF = mybir.ActivationFunctionType
ALU = mybir.AluOpType
DR = mybir.MatmulPerfMode.DoubleRow

# engine split for the 72 (slot, kt) tiles: copies ACT-heavy (gpsimd has no
# PSUM access and no max op, so the max-accumulate chain lives on DVE)
ACT_COPY_MOD = 9        # copy on ACT when (idx % MOD) < THR
ACT_COPY_THR = 7

_cache = {}


def _build():
    nc = bacc.Bacc("TRN2", target_bir_lowering=False, debug=False, num_devices=NC)

    xq_d = nc.dram_tensor("xq", [SLAB, D], F32, kind="ExternalInput")
    part_d = nc.dram_tensor("partial", [1, 1], F32, kind="ExternalOutput")
    payload_d = nc.dram_tensor("payload", [PROWS, SLAB], F8)
    shared_d = nc.dram_tensor("shared", [NC * PROWS, SLAB], F8, addr_space="Shared")

    with tile.TileContext(nc) as tc:
        with (
            tc.tile_pool(name="const", bufs=1) as constp,
            tc.tile_pool(name="big", bufs=1) as bigp,
        ):
            identf = constp.tile([P, P], F32)
            make_identity(nc, identf[:])
            ident8 = constp.tile([8, 8], F32)
            make_identity(nc, ident8[:])
            ones = constp.tile([P, 1], F32)
            nc.vector.memset(ones[:], 1.0)
            nq = constp.tile([P, QT], F32)

            xq_sb = bigp.tile([P, QT, D], F32)
            xTq = bigp.tile([P, KC, SLAB], F8)
            xTc = [bigp.tile([P, KC, SLAB], F8, name=f"xTc{c}") for c in range(NC)]
            m_runA = bigp.tile([P, SLAB], F16)
            negdn = constp.tile([P, QT], F32)
            bias32 = [constp.tile([P, QT], F32, name=f"bias{c}") for c in range(NC)]

            with (
                tc.tile_pool(name="wpsum", bufs=1, space="PSUM") as wpsum,
                tc.tile_pool(name="small", bufs=1) as smallp,
            ):
                # ---- load own slab, norms, transpose own slab to fp8 ----
                for qt in range(QT):
                    nc.sync.dma_start(
                        out=xq_sb[:, qt, :], in_=xq_d.ap()[qt * P : (qt + 1) * P]
                    )
                for qt in range(QT):
                    sq = smallp.tile([P, D], F32, tag="sq", bufs=2)
                    nc.scalar.activation(
                        out=sq[:], in_=xq_sb[:, qt, :], func=AF.Square,
                        accum_out=nq[:, qt : qt + 1],
                    )
                    pt = wpsum.tile([P, KC * P], F32, tag="tr", bufs=2)
                    for kc in range(KC):
                        nc.tensor.transpose(
                            pt[:, kc * P : (kc + 1) * P],
                            xq_sb[:, qt, kc * P : (kc + 1) * P],
                            identf[:],
                        )
                    nc.scalar.copy(
                        out=xTq[:, :, qt * P : (qt + 1) * P],
                        in_=pt[:].rearrange("p (kc q) -> p kc q", kc=KC),
                    )

                # negdn = -(nq-512)/2 = nq*(-0.5) + 256 ;  fp8 split a+b
                nc.vector.tensor_scalar(
                    negdn[:], nq[:], -0.5, 256.0, op0=ALU.mult, op1=ALU.add
                )
                a8 = smallp.tile([P, QT], F8, tag="a8")
                nc.vector.tensor_copy(a8[:], negdn[:])
                a32 = smallp.tile([P, QT], F32, tag="a32")
                nc.vector.tensor_copy(a32[:], a8[:])
                b32 = smallp.tile([P, QT], F32, tag="b32")
                nc.vector.tensor_tensor(
                    out=b32[:], in0=negdn[:], in1=a8[:], op=ALU.subtract
                )
                # transpose a/b [128, 8] -> [8, 128] for a contiguous DRAM write
                abT_ps = wpsum.tile([8, 2, P], F32, tag="tr", bufs=2)
                nc.tensor.transpose(abT_ps[:, 0, :], a32[:], identf[:])
                nc.tensor.transpose(abT_ps[:, 1, :], b32[:], identf[:])
                abT = smallp.tile([8, 2, P], F8, tag="abT")
                nc.scalar.copy(out=abT[:], in_=abT_ps[:])

                # ---- payload -> DRAM, AllGather on the sync engine ----
                nc.sync.dma_start(
                    out=payload_d.ap()[0:D].rearrange("(kc p) q -> p kc q", p=P),
                    in_=xTq[:],
                )
                nc.sync.dma_start(
                    out=payload_d.ap()[D : D + 2].rearrange(
                        "t (qt p) -> qt t p", p=P
                    ),
                    in_=abT[:].rearrange("qt t p -> qt t p"),
                )
                # init running-max chain
                nc.vector.memset(m_runA[:], -10000.0)

                # AllGather on gpsimd; gpsimd then stalls until it completes,
                # so everything pre-AG avoids that engine.
                nc.gpsimd.collective_compute(
                    "AllGather",
                    mybir.AluOpType.bypass,
                    replica_groups=[list(range(NC))],
                    ins=[payload_d.ap().opt()],
                    outs=[shared_d.ap().opt()],
                )

                idx = 0

                def score_tiles(keys, bias):
                    nonlocal idx
                    for kt in range(QT):
                        pp = wpsum.tile([P, SLAB], F32, tag="pp", bufs=2)
                        for c2 in range(2):
                            for qh in range(2):
                                nc.tensor.matmul(
                                    pp[:, qh * 512 : (qh + 1) * 512],
                                    lhsT=keys[:, 2 * c2 : 2 * c2 + 2, kt * P : (kt + 1) * P],
                                    rhs=xTq[:, 2 * c2 : 2 * c2 + 2, qh * 512 : (qh + 1) * 512],
                                    start=(c2 == 0),
                                    stop=(c2 == 1),
                                    perf_mode=DR,
                                )
                        s16 = smallp.tile([P, SLAB], F16, tag="s16", bufs=6)
                        bap = bias[:, kt : kt + 1]
                        if (idx % ACT_COPY_MOD) < ACT_COPY_THR:
                            nc.scalar.add(s16[:], pp[:], bap)
                        else:
                            nc.vector.tensor_scalar_add(s16[:], pp[:], bap)
                        nc.vector.tensor_tensor(
                            out=m_runA[:], in0=s16[:], in1=m_runA[:], op=ALU.max
                        )
                        idx += 1

                # ---- slot 0: own slab (overlaps the AllGather) ----
                score_tiles(xTq, negdn)

                # ---- fetch AG chunks, then their score tiles ----
                for c in range(NC):
                    base = c * PROWS
                    nc.sync.dma_start(
                        out=xTc[c][:],
                        in_=shared_d.ap()[base : base + D].rearrange(
                            "(kc p) q -> p kc q", p=P
                        ),
                    )
                    abc = smallp.tile([8, 2, P], F8, tag="abc", bufs=2)
                    nc.sync.dma_start(
                        out=abc[:],
                        in_=shared_d.ap()[base + D : base + D + 2].rearrange(
                            "t (qt p) -> qt t p", p=P
                        ),
                    )
                    abc32 = smallp.tile([8, 2, P], F32, tag="abc32", bufs=2)
                    nc.vector.tensor_copy(abc32[:], abc[:])
                    ab_ps = wpsum.tile([P, 2, 8], F32, tag="tr", bufs=2)
                    nc.tensor.transpose(ab_ps[:, 0, :], abc32[:, 0, :], ident8[:])
                    nc.tensor.transpose(ab_ps[:, 1, :], abc32[:, 1, :], ident8[:])
                    absb = smallp.tile([P, 2, 8], F32, tag="absb", bufs=2)
                    nc.scalar.copy(out=absb[:], in_=ab_ps[:])
                    nc.vector.tensor_tensor(
                        out=bias32[c][:], in0=absb[:, 0, :], in1=absb[:, 1, :],
                        op=ALU.add,
                    )
                for c in range(NC):
                    score_tiles(xTc[c], bias32[c])

                # ---- per-query top-2 across key lanes ----
                m32 = smallp.tile([P, SLAB], F32, tag="m32")
                nc.vector.tensor_copy(m32[:], m_runA[:])
                ftr = wpsum.tile([P, SLAB], F32, tag="pp", bufs=2)
                for b in range(QT):
                    nc.tensor.transpose(
                        ftr[:, b * P : (b + 1) * P],
                        m32[:, b * P : (b + 1) * P],
                        identf[:],
                    )
                mt = smallp.tile([P, QT, P], F16, tag="mt")
                nc.scalar.copy(
                    out=mt[:], in_=ftr[:].rearrange("p (b q) -> p b q", b=QT)
                )
                gtop = smallp.tile([P, QT, 8], F16, tag="gtop")
                for b in range(QT):
                    nc.vector.max(out=gtop[:, b, :], in_=mt[:, b, :])

                # rho^2 = 2*(m1 - m2), m1 = 512 - negdn (exact fp32)
                m2_32 = smallp.tile([P, QT], F32, tag="m2")
                nc.vector.tensor_copy(
                    m2_32[:], gtop[:, :, 1:2].rearrange("p b r -> p (b r)")
                )
                m1f = smallp.tile([P, QT], F32, tag="m1")
                nc.vector.tensor_scalar(
                    m1f[:], negdn[:], -1.0, 512.0, op0=ALU.mult, op1=ALU.add
                )
                delta = smallp.tile([P, QT], F32, tag="delta")
                nc.vector.tensor_tensor(
                    out=delta[:], in0=m1f[:], in1=m2_32[:], op=ALU.subtract
                )
                logs = smallp.tile([P, QT], F32, tag="logs")
                nc.scalar.activation(
                    out=logs[:], in_=delta[:], func=AF.Ln, bias=0.0, scale=2.0
                )
                rowsum = smallp.tile([P, 1], F32, tag="rowsum")
                nc.vector.tensor_reduce(
                    rowsum[:], logs[:], axis=mybir.AxisListType.X, op=ALU.add
                )
                fin = wpsum.tile([1, 1], F32, tag="fin", bufs=1)
                nc.tensor.matmul(
                    fin[:], lhsT=rowsum[:], rhs=ones[:], start=True, stop=True
                )
                outsb = smallp.tile([1, 1], F32, tag="outsb")
                nc.scalar.copy(outsb[:], fin[:])
                nc.sync.dma_start(out=part_d.ap(), in_=outsb[:])

    nc.compile()
    return nc


def get_nc():
    if "nc" not in _cache:
        _cache["nc"] = _build()
    return _cache["nc"]


def run(x: np.ndarray, **spmd_kwargs):
    nc = get_nc()
    x = np.ascontiguousarray(x, dtype=np.float32)
    in_maps = [{"xq": x[c * SLAB : (c + 1) * SLAB]} for c in range(NC)]
    res = run_bass_kernel_spmd(nc, in_maps, list(range(NC)), **spmd_kwargs)
    total = sum(float(res.results[c]["partial"][0, 0]) for c in range(NC))
    # partial = sum of log(rho^2) = sum of 2*log(rho)
    loss = np.float32(-0.5 * total / N)
    return np.asarray(loss, dtype=np.float32), res


def kernel(x: np.ndarray) -> np.ndarray:
    loss, _ = run(x)
    return loss


# revision 17
# speedup vs baseline: 1.0487x; 1.0487x over previous
"""Trainium2 Bass kernel for DifferentialEntropyRegularization (kNN loss).

reference math:
    dots = x @ x.T ; dots[i,i] = -1
    I = argmax(dots, axis=1)
    rho = ||x - x[I] + 1e-6||_2
    loss = -mean(log(rho + 1e-8))

Strategy (8 NeuronCores, fp8, hybrid AllGather + local transposes):
  - each core owns a 1024-row query slab. Keys arrive transposed-to-fp8
    from three sources: its own slab (PE transpose), N_LOCAL neighbor
    slabs passed as extra inputs and PE-transposed locally (this work
    fills the AllGather's barrier/transfer latency window), and the
    remaining slabs from an fp8 AllGather of every core's transposed
    slab. AG chunks are fetched with indirect DMA driven by per-core
    host-computed row-offset tables (SPMD-safe relative indexing).
  - payload rows carry the per-key norm correction -(n_k-512)/2 as two
    fp8 bytes appended to the kc=0 rows, so it needs no extra messages.
  - scores s[k,q] = <x_k,x_q>_fp8 - (n_k-512)/2 via fp8 DoubleRow
    matmuls with keys on partitions; the correction is a per-partition
    bias applied on the PSUM->SBUF copy (split ACT/DVE), then an f16
    running max per key-lane (DVE), then a small PE transpose + MAX8
    recovers each query's top-2 across the 128 key lanes:
    rho^2 = 2*(m1 - m2), m1 = n_q/2 + 256 exact in fp32.
  - per-core partial sums of log(rho^2) reduced on host.
"""

import sys

sys.path.insert(0, "/opt/trn_rl_repo")

import numpy as np

import concourse.bass as bass
import concourse.mybir as mybir
import concourse.tile as tile
from concourse import bacc
from concourse.bass import IndirectOffsetOnAxis
from concourse.bass_utils import run_bass_kernel_spmd
from concourse.masks import make_identity

N = 8192
D = 512
NC = 8
SLAB = N // NC          # 1024 query rows per core
P = 128                 # partitions
QT = SLAB // P          # 8 row tiles per slab
KC = D // P             # 4 contraction chunks of 128
N_LOCAL = 4             # neighbor slabs transposed locally (fills AG window)
N_AG = NC - 1 - N_LOCAL # slabs consumed from the AllGather
PCOL = SLAB + 16        # payload row: 1024 keys + 16 bias bytes (a|b)

F32 = mybir.dt.float32
F16 = mybir.dt.float16
F8 = mybir.dt.float8e4
I32 = mybir.dt.int32
U32 = mybir.dt.uint32
AF = mybir.ActivationFunctionType
ALU = mybir.AluOpType
DR = mybir.MatmulPerfMode.DoubleRow

# copy-engine split for the 64 (slot, kt) tiles
ACT_COPY_MOD = 4        # copy on ACT when (idx % MOD) < THR, else DVE
ACT_COPY_THR = 3

_cache = {}


def _build():
    nc = bacc.Bacc("TRN2", target_bir_lowering=False, debug=False, num_devices=NC)

    xq_d = nc.dram_tensor("xq", [SLAB, D], F32, kind="ExternalInput")
    xl_d = [
        nc.dram_tensor(f"xl{n}", [SLAB, D], F32, kind="ExternalInput")
        for n in range(N_LOCAL)
    ]
    offs_d = nc.dram_tensor("offs", [P, N_AG * KC], U32, kind="ExternalInput")
    part_d = nc.dram_tensor("partial", [1, 1], F32, kind="ExternalOutput")
    payload_d = nc.dram_tensor("payload", [D, PCOL], F8)
    shared_d = nc.dram_tensor("shared", [NC * D, PCOL], F8, addr_space="Shared")

    with tile.TileContext(nc) as tc:
        with (
            tc.tile_pool(name="const", bufs=1) as constp,
            tc.tile_pool(name="big", bufs=1) as bigp,
        ):
            identf = constp.tile([P, P], F32)
            make_identity(nc, identf[:])
            ones = constp.tile([P, 1], F32)
            nc.vector.memset(ones[:], 1.0)
            nq = constp.tile([P, QT], F32)
            negdn = constp.tile([P, QT], F32)
            offs_sb = constp.tile([P, N_AG * KC], U32)

            xq_sb = bigp.tile([P, QT, D], F32)
            xTq = bigp.tile([P, KC, SLAB], F8)
            xTl = [bigp.tile([P, KC, SLAB], F8, name=f"xTl{n}") for n in range(N_LOCAL)]
            biasl = [constp.tile([P, QT], F32, name=f"biasl{n}") for n in range(N_LOCAL)]
            xTg = [bigp.tile([P, KC, PCOL], F8, name=f"xTg{j}") for j in range(N_AG)]
            biasg = [constp.tile([P, QT], F32, name=f"biasg{j}") for j in range(N_AG)]
            m_runA = bigp.tile([P, SLAB], F16)

            with (
                tc.tile_pool(name="wpsum", bufs=1, space="PSUM") as wpsum,
                tc.tile_pool(name="small", bufs=1) as smallp,
                tc.tile_pool(name="xlp", bufs=2) as xlp,
            ):
                # ---- own slab: load, norms, transpose to fp8 ----
                for qt in range(QT):
                    nc.sync.dma_start(
                        out=xq_sb[:, qt, :], in_=xq_d.ap()[qt * P : (qt + 1) * P]
                    )
                nc.sync.dma_start(out=offs_sb[:], in_=offs_d.ap())

                def slab_transpose(src_sb, dst_f8):
                    for qt in range(QT):
                        pt = wpsum.tile([P, KC * P], F32, tag="tr", bufs=2)
                        for kc in range(KC):
                            nc.tensor.transpose(
                                pt[:, kc * P : (kc + 1) * P],
                                src_sb[:, qt, kc * P : (kc + 1) * P],
                                identf[:],
                            )
                        nc.scalar.copy(
                            out=dst_f8[:, :, qt * P : (qt + 1) * P],
                            in_=pt[:].rearrange("p (kc q) -> p kc q", kc=KC),
                        )

                def slab_norms(src_sb, nq_out, negdn_out):
                    # nq = ||row||^2 (fp32), negdn = -(nq-512)/2
                    for qt in range(QT):
                        sq = smallp.tile([P, D], F32, tag="sq", bufs=2)
                        nc.scalar.activation(
                            out=sq[:], in_=src_sb[:, qt, :], func=AF.Square,
                            accum_out=nq_out[:, qt : qt + 1],
                        )
                    nc.vector.tensor_scalar(
                        negdn_out[:], nq_out[:], -0.5, 256.0,
                        op0=ALU.mult, op1=ALU.add,
                    )

                slab_norms(xq_sb, nq, negdn)
                slab_transpose(xq_sb, xTq)

                # fp8 a+b split of negdn -> 16 bias bytes on the kc=0 rows
                a8 = smallp.tile([P, QT], F8, tag="a8")
                nc.vector.tensor_copy(a8[:], negdn[:])
                b32 = smallp.tile([P, QT], F32, tag="b32")
                nc.vector.tensor_tensor(
                    out=b32[:], in0=negdn[:], in1=a8[:], op=ALU.subtract
                )
                ab16 = smallp.tile([P, 2, QT], F8, tag="ab16")
                nc.vector.tensor_copy(ab16[:, 0, :], a8[:])
                nc.vector.tensor_copy(ab16[:, 1, :], b32[:])

                # ---- payload -> DRAM, AllGather (gpsimd stalls on it) ----
                nc.sync.dma_start(
                    out=payload_d.ap()[0:D, 0:SLAB].rearrange(
                        "(kc p) q -> p kc q", p=P
                    ),
                    in_=xTq[:],
                )
                nc.sync.dma_start(
                    out=payload_d.ap()[0:P, SLAB:PCOL], in_=ab16[:]
                )
                nc.gpsimd.collective_compute(
                    "AllGather",
                    mybir.AluOpType.bypass,
                    replica_groups=[list(range(NC))],
                    ins=[payload_d.ap().opt()],
                    outs=[shared_d.ap().opt()],
                )

                nc.vector.memset(m_runA[:], -10000.0)

                idx = 0

                def score_tiles(keys, bias):
                    nonlocal idx
                    for kt in range(QT):
                        pp = wpsum.tile([P, SLAB], F32, tag="pp", bufs=3)
                        for c2 in range(2):
                            for qh in range(2):
                                nc.tensor.matmul(
                                    pp[:, qh * 512 : (qh + 1) * 512],
                                    lhsT=keys[:, 2 * c2 : 2 * c2 + 2, kt * P : (kt + 1) * P],
                                    rhs=xTq[:, 2 * c2 : 2 * c2 + 2, qh * 512 : (qh + 1) * 512],
                                    start=(c2 == 0),
                                    stop=(c2 == 1),
                                    perf_mode=DR,
                                )
                        s16 = smallp.tile([P, SLAB], F16, tag="s16", bufs=6)
                        bap = bias[:, kt : kt + 1]
                        if (idx % ACT_COPY_MOD) < ACT_COPY_THR:
                            nc.scalar.add(s16[:], pp[:], bap)
                        else:
                            nc.vector.tensor_scalar_add(s16[:], pp[:], bap)
                        nc.vector.tensor_tensor(
                            out=m_runA[:], in0=s16[:], in1=m_runA[:], op=ALU.max
                        )
                        idx += 1

                # ---- slot 0: own slab (inside the AG window) ----
                score_tiles(xTq, negdn)

                # ---- local neighbor slabs: load+transpose+norms, then MMs ----
                for n in range(N_LOCAL):
                    xl_sb = xlp.tile([P, QT, D], F32, tag="xl")
                    eng = nc.sync if n % 2 == 0 else nc.scalar
                    for qt in range(QT):
                        eng.dma_start(
                            out=xl_sb[:, qt, :],
                            in_=xl_d[n].ap()[qt * P : (qt + 1) * P],
                        )
                    nql = smallp.tile([P, QT], F32, tag="nql", bufs=2)
                    slab_norms(xl_sb, nql, biasl[n])
                    slab_transpose(xl_sb, xTl[n])
                    score_tiles(xTl[n], biasl[n])

                # ---- AG slabs via indirect DMA (per-core offset tables) ----
                for j in range(N_AG):
                    for kc in range(KC):
                        col = j * KC + kc
                        nc.gpsimd.indirect_dma_start(
                            out=xTg[j][:, kc, :],
                            out_offset=None,
                            in_=shared_d.ap(),
                            in_offset=IndirectOffsetOnAxis(
                                ap=offs_sb[:, col : col + 1], axis=0
                            ),
                        )
                    nc.vector.tensor_tensor(
                        out=biasg[j][:],
                        in0=xTg[j][:, 0, SLAB : SLAB + QT],
                        in1=xTg[j][:, 0, SLAB + QT : SLAB + 2 * QT],
                        op=ALU.add,
                    )
                for j in range(N_AG):
                    score_tiles(xTg[j], biasg[j])

                # ---- per-query top-2 across key lanes ----
                m32 = smallp.tile([P, SLAB], F32, tag="m32")
                nc.vector.tensor_copy(m32[:], m_runA[:])
                ftr = wpsum.tile([P, SLAB], F32, tag="pp", bufs=3)
                for b in range(QT):
                    nc.tensor.transpose(
                        ftr[:, b * P : (b + 1) * P],
                        m32[:, b * P : (b + 1) * P],
                        identf[:],
                    )
                mt = smallp.tile([P, QT, P], F16, tag="mt")
                nc.scalar.copy(
                    out=mt[:], in_=ftr[:].rearrange("p (b q) -> p b q", b=QT)
                )
                gtop = smallp.tile([P, QT, 8], F16, tag="gtop")
                for b in range(QT):
                    nc.vector.max(out=gtop[:, b, :], in_=mt[:, b, :])

                # rho^2 = 2*(m1 - m2), m1 = 512 - negdn (exact fp32)
                m2_32 = smallp.tile([P, QT], F32, tag="m2")
                nc.vector.tensor_copy(
                    m2_32[:], gtop[:, :, 1:2].rearrange("p b r -> p (b r)")
                )
                m1f = smallp.tile([P, QT], F32, tag="m1")
                nc.vector.tensor_scalar(
                    m1f[:], negdn[:], -1.0, 512.0, op0=ALU.mult, op1=ALU.add
                )
                delta = smallp.tile([P, QT], F32, tag="delta")
                nc.vector.tensor_tensor(
                    out=delta[:], in0=m1f[:], in1=m2_32[:], op=ALU.subtract
                )
                logs = smallp.tile([P, QT], F32, tag="logs")
                nc.scalar.activation(
                    out=logs[:], in_=delta[:], func=AF.Ln, bias=0.0, scale=2.0
                )
                rowsum = smallp.tile([P, 1], F32, tag="rowsum")
                nc.vector.tensor_reduce(
                    rowsum[:], logs[:], axis=mybir.AxisListType.X, op=ALU.add
                )
                fin = wpsum.tile([1, 1], F32, tag="tr", bufs=2)
                nc.tensor.matmul(
                    fin[:], lhsT=rowsum[:], rhs=ones[:], start=True, stop=True
                )
                outsb = smallp.tile([1, 1], F32, tag="outsb")
                nc.scalar.copy(outsb[:], fin[:])
                nc.sync.dma_start(out=part_d.ap(), in_=outsb[:])

    nc.compile()
    return nc


def get_nc():
    if "nc" not in _cache:
        _cache["nc"] = _build()
    return _cache["nc"]


def run(x: np.ndarray, **spmd_kwargs):
    nc = get_nc()
    x = np.ascontiguousarray(x, dtype=np.float32)
    in_maps = []
    for c in range(NC):
        m = {"xq": x[c * SLAB : (c + 1) * SLAB]}
        for n in range(N_LOCAL):
            src = (c + 1 + n) % NC
            m[f"xl{n}"] = x[src * SLAB : (src + 1) * SLAB]
        offs = np.empty((P, N_AG * KC), np.uint32)
        p = np.arange(P, dtype=np.uint32)
        for j in range(N_AG):
            src = (c + 1 + N_LOCAL + j) % NC
            for kc in range(KC):
                offs[:, j * KC + kc] = src * D + kc * P + p
        m["offs"] = offs
        in_maps.append(m)
    res = run_bass_kernel_spmd(nc, in_maps, list(range(NC)), **spmd_kwargs)
    total = sum(float(res.results[c]["partial"][0, 0]) for c in range(NC))
    # partial = sum of log(rho^2) = sum of 2*log(rho)
    loss = np.float32(-0.5 * total / N)
    return np.asarray(loss, dtype=np.float32), res


def kernel(x: np.ndarray) -> np.ndarray:
    loss, _ = run(x)
    return loss


# revision 21
# speedup vs baseline: 1.1784x; 1.1236x over previous
"""Trainium2 Bass kernel for DifferentialEntropyRegularization (kNN loss).

reference math:
    dots = x @ x.T ; dots[i,i] = -1
    I = argmax(dots, axis=1)
    rho = ||x - x[I] + 1e-6||_2
    loss = -mean(log(rho + 1e-8))

Strategy (8 NeuronCores, data-parallel over rows of x, no cross-core sync):
  - each core owns a 1024-row slab of queries; keys = all 8192 rows.
  - x is replicated; every core PE-transposes all of x locally from fp32
    (fp8 cast happens inside the PSUM->SBUF copy), interleaved just-in-time
    with the first query tiles; row loads spread over 2 engine DMA queues.
  - dots via fp8e4m3 DoubleRow matmuls (fp32 PSUM accumulation). Top-1 of
    every row is the self-dot (~512 >> max cross-dot ~90), so no diagonal
    masking: the top-2 is the nearest neighbor.
  - two-level argmax: per 1024-key pair-block, MAX8 on the fp16 SBUF copy ->
    per-pair top8; rank-major top-2-per-pair view -> global top8 + winning
    pair id; the winning pair row is fetched back from a DRAM copy of the
    dots (indirect DMA) and FIND_INDEX8 recovers the key index within it.
  - rho computed exactly in fp32 from gathered x[j*] rows (indirect DMA),
    identical arithmetic to the reference; only argmax selection is fp8/fp16.
  - per-core partial sums of log(rho+eps) reduced on host.
"""

import sys

sys.path.insert(0, "/opt/trn_rl_repo")

import numpy as np

import concourse.bass as bass
import concourse.mybir as mybir
import concourse.tile as tile
from concourse import bacc
from concourse.bass import IndirectOffsetOnAxis
from concourse.bass_utils import run_bass_kernel_spmd
from concourse.masks import make_identity

N = 8192
D = 512
NC = 8
SLAB = N // NC          # 1024 query rows per core
P = 128                 # partitions
QT = SLAB // P          # 8 query tiles per core
NB = 512                # key block (free dim per matmul)
KB = N // NB            # 16 key blocks
KC = D // P             # 4 contraction chunks
NP = NC                 # 8 pair-blocks (1024 keys each)

F32 = mybir.dt.float32
BF16 = mybir.dt.bfloat16
F8 = mybir.dt.float8e4
F16 = mybir.dt.float16
U32 = mybir.dt.uint32
AF = mybir.ActivationFunctionType
ALU = mybir.AluOpType

_cache = {}


def _build():
    nc = bacc.Bacc("TRN2", target_bir_lowering=False, debug=False, num_devices=NC)

    x_d = nc.dram_tensor("x", [N, D], F32, kind="ExternalInput")
    xq_d = nc.dram_tensor("xq", [SLAB, D], F32, kind="ExternalInput")
    part_d = nc.dram_tensor("partial", [1, 1], F32, kind="ExternalOutput")
    # per-qt DRAM copy of the dots; row = pair*P + p holds a 1024-key pair
    dotsd = [nc.dram_tensor(f"dotsd{qt}", [NP * P, 2 * NB], F16) for qt in range(QT)]

    with tile.TileContext(nc) as tc:
        with (
            tc.tile_pool(name="const", bufs=1) as constp,
            tc.tile_pool(name="big", bufs=1) as bigp,
        ):
            identf = constp.tile([P, P], F32)
            make_identity(nc, identf[:])
            ones = constp.tile([P, 1], F32)
            nc.vector.memset(ones[:], 1.0)
            eps_pd = constp.tile([P, 1], F32)
            nc.vector.memset(eps_pd[:], 1e-6)
            eps_log = constp.tile([P, 1], F32)
            nc.vector.memset(eps_log[:], 1e-8)
            piota = constp.tile([P, 1], F32)
            nc.gpsimd.iota(
                piota[:], pattern=[[0, 1]], base=0, channel_multiplier=1,
                allow_small_or_imprecise_dtypes=True,
            )
            logs = constp.tile([P, QT], F32)

            # own slab, fp32, tiled [p, qt, d]
            xq_sb = bigp.tile([P, QT, D], F32)
            for qt in range(QT):
                nc.sync.dma_start(
                    out=xq_sb[:, qt, :], in_=xq_d.ap()[qt * P : (qt + 1) * P]
                )

            # transposed own slab (fp8): [p=d-chunk, kc, query]
            xTq = bigp.tile([P, KC, SLAB], F8)
            # full transposed keys (fp8), one tile per 1024-key chunk
            xTc = [bigp.tile([P, KC, SLAB], F8, name=f"xTc{c}") for c in range(NC)]
            # gathered nearest-neighbor rows per qt
            nn_rows = bigp.tile([P, QT, D], F32)

            with (
                tc.tile_pool(name="wpsum", bufs=3, space="PSUM") as wpsum,
                tc.tile_pool(name="small", bufs=3) as smallp,
            ):
                # ---- own-slab transpose (query lhsT), fp32 -> fp8 in copy ----
                for qt in range(QT):
                    pt = wpsum.tile([P, KC * P], F32, tag="work")
                    for kc in range(KC):
                        nc.tensor.transpose(
                            pt[:, kc * P : (kc + 1) * P],
                            xq_sb[:, qt, kc * P : (kc + 1) * P],
                            identf[:],
                        )
                    nc.scalar.copy(
                        out=xTq[:, :, qt * P : (qt + 1) * P],
                        in_=pt[:].rearrange("p (kc q) -> p kc q", kc=KC),
                    )

                # ---- key-chunk prep: load x rows (2 row-tiles per step),
                # cast bf16, PE transpose, one wide SBUF copy ----
                load_engines = [nc.sync, nc.gpsimd]

                def prep_chunk(c):
                    for t in range(0, QT, 2):  # 2 row tiles of 128 per step
                        g = c * QT + t
                        xf = smallp.tile([P, 2, D], F32, tag="xf", bufs=6)
                        load_engines[(g // 2) % 2].dma_start(
                            out=xf[:],
                            in_=x_d.ap()[g * P : (g + 2) * P].rearrange(
                                "(t p) d -> p t d", p=P
                            ),
                        )
                        pt = wpsum.tile([P, 2 * KC * P], F32, tag="work")
                        for tt in range(2):
                            for kc in range(KC):
                                nc.tensor.transpose(
                                    pt[:, (tt * KC + kc) * P : (tt * KC + kc + 1) * P],
                                    xf[:, tt, kc * P : (kc + 1) * P],
                                    identf[:],
                                )
                        nc.scalar.copy(
                            out=xTc[c][:, :, t * P : (t + 2) * P].rearrange(
                                "p kc (t q) -> p t kc q", t=2
                            ),
                            in_=pt[:].rearrange(
                                "p (t kc q) -> p t kc q", t=2, kc=KC
                            ),
                        )

                rho2 = smallp.tile([P, QT], F32, tag="rho2", bufs=1)
                EARLY = 5  # query tiles interleaved with the key prep/load
                btops = {}

                def mm_pair(qt, pr):
                    pp = wpsum.tile([P, 2 * NB], F32, tag="work")
                    for half in range(2):
                        for kc2 in range(KC // 2):
                            nc.tensor.matmul(
                                pp[:, half * NB : (half + 1) * NB],
                                lhsT=xTq[:, 2 * kc2 : 2 * kc2 + 2, qt * P : (qt + 1) * P],
                                rhs=xTc[pr][:, 2 * kc2 : 2 * kc2 + 2, half * NB : (half + 1) * NB],
                                start=(kc2 == 0),
                                stop=(kc2 == KC // 2 - 1),
                                perf_mode=mybir.MatmulPerfMode.DoubleRow,
                            )
                    # PSUM -> SBUF pair copy (one wide ACT copy), then -> DRAM + top8
                    dcopy = smallp.tile([P, 2 * NB], F16, tag="dcopy", bufs=6)
                    nc.scalar.copy(out=dcopy[:], in_=pp[:])
                    eng = nc.sync if (pr % 2 == 0) else nc.gpsimd
                    eng.dma_start(
                        out=dotsd[qt].ap()[pr * P : (pr + 1) * P], in_=dcopy[:]
                    )
                    nc.vector.max(out=btops[qt][:, pr, :], in_=dcopy[:])

                def qt_chain(qt):
                    btop = btops[qt]
                    # rank-major top-2-per-pair: btop2[:, r*NP + pr]
                    btop2 = smallp.tile([P, 2 * NP], F16, tag="btop2")
                    for r in range(2):
                        nc.vector.tensor_copy(btop2[:, r * NP : (r + 1) * NP], btop[:, :, r])
                    gtop = smallp.tile([P, 8], F16, tag="gtop")
                    nc.vector.max(out=gtop[:], in_=btop2[:])
                    pos8 = smallp.tile([P, 8], U32, tag="pos8")
                    nc.vector.max_index(out=pos8[:], in_max=gtop[:], in_values=btop2[:])

                    # pos2 in [0, 16); pair = pos2 mod 8 (fp32 math, exact)
                    pos_f = smallp.tile([P, 1], F32, tag="pos_f")
                    nc.vector.tensor_copy(pos_f[:], pos8[:, 1:2])
                    tmp = smallp.tile([P, 1], F32, tag="tmp")
                    nc.vector.tensor_scalar(
                        tmp[:], pos_f[:], float(NP), float(NP), op0=ALU.is_ge, op1=ALU.mult
                    )
                    b_f = smallp.tile([P, 1], F32, tag="b_f")
                    nc.vector.tensor_tensor(
                        out=b_f[:], in0=pos_f[:], in1=tmp[:], op=ALU.subtract
                    )
                    # gidx = pair*128 + p  (row into dotsd[qt])
                    gidx_f = smallp.tile([P, 1], F32, tag="gidx_f")
                    nc.vector.tensor_scalar(
                        gidx_f[:], b_f[:], float(P), piota[:], op0=ALU.mult, op1=ALU.add
                    )
                    gidx = smallp.tile([P, 1], U32, tag="gidx")
                    nc.vector.tensor_copy(gidx[:], gidx_f[:])

                    # fetch winning pair row per query, find v2's column in it
                    dblk = smallp.tile([P, 2 * NB], F16, tag="dblk")
                    nc.gpsimd.indirect_dma_start(
                        out=dblk[:],
                        out_offset=None,
                        in_=dotsd[qt].ap(),
                        in_offset=IndirectOffsetOnAxis(ap=gidx[:, :1], axis=0),
                    )
                    l8 = smallp.tile([P, 8], U32, tag="l8")
                    nc.vector.max_index(out=l8[:], in_max=gtop[:], in_values=dblk[:])

                    # j* = pair*1024 + l
                    l_f = smallp.tile([P, 1], F32, tag="l_f")
                    nc.vector.tensor_copy(l_f[:], l8[:, 1:2])
                    j_f = smallp.tile([P, 1], F32, tag="j_f")
                    nc.vector.tensor_scalar(
                        j_f[:], b_f[:], float(2 * NB), l_f[:], op0=ALU.mult, op1=ALU.add
                    )
                    jst = smallp.tile([P, 1], U32, tag="jst")
                    nc.vector.tensor_copy(jst[:], j_f[:])

                    nc.gpsimd.indirect_dma_start(
                        out=nn_rows[:, qt, :],
                        out_offset=None,
                        in_=x_d.ap(),
                        in_offset=IndirectOffsetOnAxis(ap=jst[:, :1], axis=0),
                    )
                    diff = smallp.tile([P, D], F32, tag="diff")
                    nc.gpsimd.tensor_tensor(
                        out=diff[:], in0=xq_sb[:, qt, :], in1=nn_rows[:, qt, :],
                        op=ALU.subtract,
                    )
                    sq = smallp.tile([P, D], F32, tag="sq")
                    nc.scalar.activation(
                        out=sq[:],
                        in_=diff[:],
                        func=AF.Square,
                        bias=eps_pd[:],
                        scale=1.0,
                        accum_out=rho2[:, qt : qt + 1],
                    )

                # phase 1: key prep + the first EARLY query tiles, chunk-major
                for qt in range(EARLY):
                    btops[qt] = smallp.tile(
                        [P, NP, 8], F16, tag="btop", bufs=EARLY + 1, name=f"btop{qt}"
                    )
                for pr in range(NP):
                    prep_chunk(pr)
                    for qt in range(EARLY):
                        mm_pair(qt, pr)
                for qt in range(EARLY):
                    qt_chain(qt)

                # phase 2: remaining query tiles, dense
                for qt in range(EARLY, QT):
                    btops[qt] = smallp.tile(
                        [P, NP, 8], F16, tag="btop", bufs=EARLY + 1, name=f"btop{qt}"
                    )
                    for pr in range(NP):
                        mm_pair(qt, pr)
                    qt_chain(qt)

                # batched tail: rho and log for all qt at once
                rho = smallp.tile([P, QT], F32, tag="rho")
                nc.scalar.sqrt(rho[:], rho2[:])
                nc.scalar.activation(
                    out=logs[:], in_=rho[:], func=AF.Ln, bias=eps_log[:], scale=1.0
                )

                rowsum = smallp.tile([P, 1], F32, tag="rowsum")
                nc.vector.tensor_reduce(
                    rowsum[:], logs[:], axis=mybir.AxisListType.X, op=ALU.add
                )
                fin = wpsum.tile([1, 1], F32, tag="fin", bufs=1)
                nc.tensor.matmul(fin[:], lhsT=rowsum[:], rhs=ones[:], start=True, stop=True)
                outsb = smallp.tile([1, 1], F32, tag="outsb")
                nc.scalar.copy(outsb[:], fin[:])
                nc.sync.dma_start(out=part_d.ap(), in_=outsb[:])

    nc.compile()
    return nc


def get_nc():
    if "nc" not in _cache:
        _cache["nc"] = _build()
    return _cache["nc"]


def run(x: np.ndarray, **spmd_kwargs):
    nc = get_nc()
    x = np.ascontiguousarray(x, dtype=np.float32)
    in_maps = [
        {"x": x, "xq": x[c * SLAB : (c + 1) * SLAB]} for c in range(NC)
    ]
    res = run_bass_kernel_spmd(nc, in_maps, list(range(NC)), **spmd_kwargs)
    total = sum(float(res.results[c]["partial"][0, 0]) for c in range(NC))
    loss = np.float32(-total / N)
    return np.asarray(loss, dtype=np.float32), res


def kernel(x: np.ndarray) -> np.ndarray:
    loss, _ = run(x)
    return loss

